# revision 1
# baseline (speedup 1.0000x reference)
"""EnergyBasedVAD Trainium2 kernel.

Input:  waveform (32, 960000) f32.
Output: (32, 3749) bool VAD mask.

Sharding: pure data parallel — 4 batch rows per core across 8 cores.

Device computes short-time energy (the memory-bound part: 123 MB of
waveform reads). Each row of 960000 samples is loaded as 125 partitions
x 7936 samples (stride 7680 — each partition re-reads the next one's
first 256 samples so all 30 frame windows live within one partition),
squared with the 1/512 mean folded into the activation scale, then
block-summed 64 -> 256 -> adjacent-pair into the 3750 frame energies.
The per-core input carries 256 padding samples so the overlapping DMA
stays in bounds on the last row; frame 3749 is garbage and discarded.

Host computes the 20%-quantile threshold and the hysteresis segment
state machine on the (32, 3749) energies — 0.01% of the bytes.
"""

import math
import numpy as np

import concourse.bass as bass
import concourse.bacc as bacc
import concourse.mybir as mybir
from concourse.bass_utils import run_bass_kernel_spmd
from concourse.tile import TileContext

N_CORES = 8
B, S = 32, 960000
ROWS = B // N_CORES          # 4 rows per core
PV = 125                     # partitions holding valid data per row
P = 128                      # tile partitions (full width: 16 SBUF DMA ports)
SEG = 7680                   # samples owned per partition (30 blocks of 256)
HALO = 256
W = SEG + HALO               # 7936 samples per partition incl. halo
NBLK = S // 256              # 3750 block sums per row
T = (S - 512) // 256 + 1     # 3749 output frames
# per-core input: 4 rows + pad so partitions 125-127 of the last row's
# 128-partition load stay in bounds (their output is discarded)
FLAT = ROWS * S + (P - PV) * SEG + HALO
PSPLIT = 64                  # partition split for DMA/ACT halves (quadrant-aligned)

SILENCE_FRAMES = 18
MIN_SPEECH_FRAMES = 6
ENERGY_THRESHOLD = 0.01

_CACHE = {}


def _build(repeat: int = 1):
    nc = bacc.Bacc(None)
    wav = nc.declare_dram_parameter("waveform", [FLAT], mybir.dt.float32, isOutput=False)
    eout = nc.declare_dram_parameter("energy", [ROWS, NBLK], mybir.dt.float32, isOutput=True)

    inv = 1.0 / math.sqrt(512.0)
    sq_t = mybir.ActivationFunctionType.Square

    with TileContext(nc) as tc:
        with (
            tc.tile_pool(name="wav", bufs=3) as wav_pool,
            tc.tile_pool(name="sq", bufs=2) as sq_pool,
            tc.tile_pool(name="c64", bufs=3) as c64_pool,
            tc.tile_pool(name="c256", bufs=3) as c256_pool,
            tc.tile_pool(name="e", bufs=3) as e_pool,
        ):
            for i in range(ROWS * repeat):
                r = i % ROWS
                # alternate the two HWDGE rings (SP / Activation sequencers)
                eng = nc.sync if i % 2 == 0 else nc.scalar
                wt = wav_pool.tile([P, W], mybir.dt.float32)
                # one overlapping-stride load: partition p <- flat[r*S + p*SEG : +W]
                eng.dma_start(out=wt[:], in_=bass.AP(wav, r * S, [[SEG, P], [1, W]]))

                sq = sq_pool.tile([P, W], mybir.dt.float32)
                nc.scalar.activation(sq[:], wt[:], sq_t, scale=inv)

                # block sums: 64 -> 256 -> adjacent pair (reads SBUF once and is
                # more accurate than a flat 512-window sum)
                c64 = c64_pool.tile([P, W // 64], mybir.dt.float32)     # [128, 124]
                nc.vector.reduce_sum(
                    c64[:], sq[:].rearrange("p (n f) -> p n f", f=64),
                    axis=mybir.AxisListType.X,
                )
                c256 = c256_pool.tile([P, W // 256], mybir.dt.float32)  # [125, 31]
                nc.vector.reduce_sum(
                    c256[:], c64[:].rearrange("p (n f) -> p n f", f=4),
                    axis=mybir.AxisListType.X,
                )
                et = e_pool.tile([P, SEG // 256], mybir.dt.float32)     # [125, 30]
                nc.vector.tensor_add(et[:], c256[:, 0:30], c256[:, 1:31])

                eng.dma_start(
                    out=eout[r].rearrange("(p x) -> p x", p=PV), in_=et[0:PV, :]
                )
    nc.finalize()   # Bacc: runs the bacc compile pipeline (wait splitting, regalloc)
    return nc


def _in_maps(waveform: np.ndarray):
    w = np.ascontiguousarray(waveform, dtype=np.float32)
    wpad = np.concatenate([w.ravel(), np.zeros(FLAT - ROWS * S, np.float32)])
    return [
        {"waveform": wpad[c * ROWS * S: c * ROWS * S + FLAT]} for c in range(N_CORES)
    ]


def _run_device(waveform: np.ndarray, trace: bool = False):
    if "nc" not in _CACHE:
        _CACHE["nc"] = _build()
    nc = _CACHE["nc"]
    res = run_bass_kernel_spmd(nc, _in_maps(waveform), core_ids=list(range(N_CORES)), trace=trace)
    energy = np.concatenate([res.results[c]["energy"] for c in range(N_CORES)], axis=0)
    return energy[:, :T], res


def _vad_from_energy(e: np.ndarray) -> np.ndarray:
    """Threshold + hysteresis state machine, faithful to the reference."""
    n = e.shape[1]
    out = np.zeros((e.shape[0], n), dtype=bool)
    for b in range(e.shape[0]):
        s = np.sort(e[b])
        nzero = int((s <= 0).sum())
        nz = n - nzero
        if nz > 0:
            pos = np.float32(0.2) * np.float32(nz - 1)
            lo = int(np.floor(pos))
            hi = int(np.ceil(pos))
            frac = np.float32(pos) - np.float32(lo)
            ilo = min(max(nzero + lo, 0), n - 1)
            ihi = min(max(nzero + hi, 0), n - 1)
            thr = np.float32(s[ilo] * (np.float32(1.0) - frac) + s[ihi] * frac)
        else:
            thr = np.float32(ENERGY_THRESHOLD)
        m = e[b] > thr
        t = np.nonzero(m)[0]
        if len(t) == 0:
            continue
        grp = np.concatenate([[0], (np.diff(t) > SILENCE_FRAMES).cumsum()])
        for g in range(grp[-1] + 1):
            tg = t[grp == g]
            first, last = int(tg[0]), int(tg[-1])
            if last >= n - SILENCE_FRAMES:
                st, en = first, n      # trailing open segment
            else:
                st, en = first, last   # closed: end excludes last speech frame
            if en - st >= MIN_SPEECH_FRAMES:
                out[b, st:en] = True
    return out


def kernel(waveform: np.ndarray, _trace: bool = False) -> np.ndarray:
    energy, res = _run_device(waveform, trace=_trace)
    _CACHE["last_result"] = res
    return _vad_from_energy(energy)


# ---------------- timing utilities (test-only, not used by kernel()) ----------


def _wall_per_call(nc, in_maps, warmup=2, iters=12):
    """Min/median wall time of one device dispatch with device-resident
    inputs and no donation — mirrors run_bass_via_pjrt's shard_map path."""
    import time
    import jax
    from jax.sharding import Mesh, PartitionSpec
    from jax.experimental.shard_map import shard_map
    from concourse import bass2jax

    bass2jax.install_neuronx_cc_hook()
    n_cores = len(in_maps)
    part_name = nc.partition_id_tensor.name if nc.partition_id_tensor else None
    in_names, out_names, out_avals, zero_outs = [], [], [], []
    for alloc in nc.m.functions[0].allocations:
        if not isinstance(alloc, mybir.MemoryLocationSet):
            continue
        name = alloc.memorylocations[0].name
        if alloc.kind == "ExternalInput":
            if name != part_name:
                in_names.append(name)
        elif alloc.kind == "ExternalOutput":
            shape = tuple(alloc.tensor_shape)
            dtype = mybir.dt.np(alloc.dtype)
            out_names.append(name)
            out_avals.append(jax.core.ShapedArray(shape, dtype))
            zero_outs.append(np.zeros(shape, dtype))
    n_params = len(in_names)
    all_in_names = in_names + out_names
    if part_name is not None:
        all_in_names = all_in_names + [part_name]

    def _body(*args):
        operands = list(args)
        if part_name is not None:
            operands.append(bass2jax.partition_id_tensor())
        outs = bass2jax._bass_exec_p.bind(
            *operands,
            out_avals=tuple(out_avals),
            in_names=tuple(all_in_names),
            out_names=tuple(out_names),
            lowering_input_output_aliases=(),
            sim_require_finite=True,
            sim_require_nnan=True,
            nc=nc,
        )
        return tuple(outs)

    devices = jax.devices()[:n_cores]
    mesh = Mesh(np.asarray(devices), ("core",))
    fn = jax.jit(shard_map(
        _body, mesh=mesh,
        in_specs=(PartitionSpec("core"),) * (n_params + len(out_names)),
        out_specs=(PartitionSpec("core"),) * len(out_names),
        check_rep=False,
    ))
    sharding = jax.sharding.NamedSharding(mesh, PartitionSpec("core"))
    concat_in = [
        jax.device_put(np.concatenate([np.asarray(in_maps[c][n]) for c in range(n_cores)], 0), sharding)
        for n in in_names
    ]
    concat_zero = [
        jax.device_put(np.zeros((n_cores * z.shape[0], *z.shape[1:]), z.dtype), sharding)
        for z in zero_outs
    ]
    args = concat_in + concat_zero
    for _ in range(warmup):
        jax.block_until_ready(fn(*args))
    times = []
    for _ in range(iters):
        t0 = time.perf_counter()
        jax.block_until_ready(fn(*args))
        times.append(time.perf_counter() - t0)
    times.sort()
    return times[0], times[len(times) // 2]


def _prepare_call(nc, in_maps):
    """Compile + stage device-resident args; returns a nullary timed callable."""
    import time
    import jax
    from jax.sharding import Mesh, PartitionSpec
    from jax.experimental.shard_map import shard_map
    from concourse import bass2jax

    bass2jax.install_neuronx_cc_hook()
    n_cores = len(in_maps)
    part_name = nc.partition_id_tensor.name if nc.partition_id_tensor else None
    in_names, out_names, out_avals, zero_outs = [], [], [], []
    for alloc in nc.m.functions[0].allocations:
        if not isinstance(alloc, mybir.MemoryLocationSet):
            continue
        name = alloc.memorylocations[0].name
        if alloc.kind == "ExternalInput":
            if name != part_name:
                in_names.append(name)
        elif alloc.kind == "ExternalOutput":
            shape = tuple(alloc.tensor_shape)
            dtype = mybir.dt.np(alloc.dtype)
            out_names.append(name)
            out_avals.append(jax.core.ShapedArray(shape, dtype))
            zero_outs.append(np.zeros(shape, dtype))
    n_params = len(in_names)
    all_in_names = in_names + out_names
    if part_name is not None:
        all_in_names = all_in_names + [part_name]

    def _body(*args):
        operands = list(args)
        if part_name is not None:
            operands.append(bass2jax.partition_id_tensor())
        return tuple(bass2jax._bass_exec_p.bind(
            *operands,
            out_avals=tuple(out_avals), in_names=tuple(all_in_names),
            out_names=tuple(out_names), lowering_input_output_aliases=(),
            sim_require_finite=True, sim_require_nnan=True, nc=nc,
        ))

    devices = jax.devices()[:n_cores]
    mesh = Mesh(np.asarray(devices), ("core",))
    fn = jax.jit(shard_map(
        _body, mesh=mesh,
        in_specs=(PartitionSpec("core"),) * (n_params + len(out_names)),
        out_specs=(PartitionSpec("core"),) * len(out_names),
        check_rep=False,
    ))
    sharding = jax.sharding.NamedSharding(mesh, PartitionSpec("core"))
    args = [
        jax.device_put(np.concatenate([np.asarray(in_maps[c][n]) for c in range(n_cores)], 0), sharding)
        for n in in_names
    ] + [
        jax.device_put(np.zeros((n_cores * z.shape[0], *z.shape[1:]), z.dtype), sharding)
        for z in zero_outs
    ]

    def call():
        t0 = time.perf_counter()
        jax.block_until_ready(fn(*args))
        return time.perf_counter() - t0
    return call


def measure_exec_ns(repeat: int = 65, rounds: int = 4, iters: int = 10, verbose: bool = True):
    """Estimate HW kernel time by differencing an N-repeat program against the
    N=1 program. Measurement rounds are interleaved A/B to cancel the slow
    drift of the tunnel dispatch overhead; min-of-all per executable."""
    w = _CACHE.get("timing_input")
    if w is None:
        w = np.random.default_rng(0).standard_normal((B, S), dtype=np.float32)
    maps = _in_maps(w)
    call1 = _prepare_call(_CACHE.setdefault("nc", _build()), maps)
    callR = _prepare_call(_CACHE.setdefault(f"nc_rep{repeat}", _build(repeat)), maps)
    call1(); callR()  # warm both (NEFF load)
    t1, tR = [], []
    for _ in range(rounds):
        t1 += [call1() for _ in range(iters)]
        tR += [callR() for _ in range(iters)]
    min1, minR = min(t1), min(tR)
    ns = (minR - min1) / (repeat - 1) * 1e9
    if verbose:
        print(f"  [timing] per-call wall min: N=1 {min1*1e3:.2f}ms, N={repeat} {minR*1e3:.2f}ms"
              f" -> body {ns:.0f} ns")
    return ns

